# revision 24
# baseline (speedup 1.0000x reference)
"""Trainium2 Bass kernel: batched 1x1-conv projection + attention-style softmax mixing.

Reference computation (per batch b):
    Wp     = head_w @ W[b]                  # [512, 128]
    scores = Hf[b].T @ Wp                   # [4096, 128]   (Hf = H reshaped [512, 4096])
    A      = softmax(scores, axis=1)        # over M=128
    C      = A @ Wp.T                       # [4096, 512]
    out[b] = C.T                            # [512, 4096] -> [512, 64, 64]

Sharding: data-parallel over batch B=32 across 8 NeuronCores (4 batches/core).

Per core 16.8MB of H in + 16.8MB of C out; the 16 SDMA engines cap at
~26GB/s each with 4KB HBM runs (~416GB/s aggregate), so the DMA stream is
the end-to-end floor: ~34MB / 416GB/s ~ 82us + ~7us fixed engine preamble.
The schedule keeps both HWDGE rings (scalar + sync) loaded symmetrically
the whole run so neither stream straggles:

  - Every H macro-tile (2MB) is split h-chunkwise: chunks 0-1 load on the
    scalar ring, chunks 2-3 on the sync ring (1MB each, 4KB runs).
  - Every C macro-tile stores as two 1MB pieces at mtile end: chunks 0-1
    on the sync ring, chunks 2-3 on the scalar ring -- full 2048-element
    rows, i.e. 4KB HBM runs (1024-wide pieces would halve the per-engine
    packet rate).  Each ring therefore carries 1MB load + 1MB store per
    mtile period; loads and stores never compete within a ring FIFO
    because a store's data is always ready well before the ring drains
    to it.  The last mtile's stores split into 512KB n-halves issued as
    soon as each half is evacuated, to shrink the final flush.
  - All matmul operands are fp16 (no fp32r anywhere): FWL stays enabled,
    LDWEIGHTS overlaps the matmul stream, and PE issue stays ~2us/subtile.
  - Softmax normalization is deferred: C_raw = E @ WpT for chunks 0-1 with
    the 1/S multiply fused into the DVE PSUM evacuation; chunks 2-3 use
    A = E*r from the gpsimd (which cannot read PSUM) and evacuate as one
    [128,1024] scalar-engine copy spanning both PSUM banks.
  - PSUM: tags sc(2 banks)+sb(2)+cc(4) = 8 banks; projections ride the
    cc rotation at batch boundaries.
"""

import numpy as np

from concourse import bacc, mybir, tile
from concourse.bass_utils import run_bass_kernel_spmd

B, HD, HH, WW = 32, 512, 64, 64
TD, M = 256, 128
N = HH * WW          # 4096
NCORES = 8
BPC = B // NCORES    # 4 batches per core
NT = 512             # n-tile (free dim per matmul, bounded by one PSUM bank)
NTL = 2048           # n-macro-tile per DMA transfer (4KB strips in HBM)
NMT = N // NTL       # 2 macro-tiles per batch
NSUB = NTL // NT     # 4 matmul subtiles per macro-tile
HC = HD // 128       # 4 h-chunks
SHIFT = 64.0         # softmax stabilization shift

F32 = mybir.dt.float32
F16 = mybir.dt.float16
BF16 = mybir.dt.bfloat16


def build_nc():
    from contextlib import ExitStack

    nc = bacc.Bacc("TRN2", target_bir_lowering=False, debug=False, num_devices=NCORES)
    Hd = nc.dram_tensor("H", [BPC, HD, N], F16, kind="ExternalInput").ap()
    Wd = nc.dram_tensor("W", [BPC, TD, M], F16, kind="ExternalInput").ap()
    hwTd = nc.dram_tensor("head_wT", [TD, HD], F16, kind="ExternalInput").ap()
    Od = nc.dram_tensor("out", [BPC, HD, N], F16, kind="ExternalOutput").ap()

    with tile.TileContext(nc) as tc, ExitStack() as ctx:
        const = ctx.enter_context(tc.tile_pool(name="const", bufs=1))
        wpool = ctx.enter_context(tc.tile_pool(name="wp", bufs=1))
        hpool = ctx.enter_context(tc.tile_pool(name="h", bufs=5))
        epool = ctx.enter_context(tc.tile_pool(name="e", bufs=2))
        apool = ctx.enter_context(tc.tile_pool(name="a", bufs=3))
        cpool = ctx.enter_context(tc.tile_pool(name="c", bufs=4))
        rpool = ctx.enter_context(tc.tile_pool(name="r", bufs=2))
        ps = ctx.enter_context(tc.tile_pool(name="ps", bufs=1, space="PSUM"))

        # e/a/wpT ride bf16: exp values span ~e^-44..e^2, far beyond fp16's
        # dynamic range (fp16 e underflows to S=0 -> 1/S=inf -> NaN), while
        # bf16 keeps the fp32 exponent.  The precision-critical scores path
        # (h, wp_flat) stays fp16.
        ones16 = const.tile([128, 128], BF16, tag="ones16")
        nc.vector.memset(ones16[:], 1.0)
        neg_shift = const.tile([128, 1], F32, tag="neg_shift")
        nc.vector.memset(neg_shift[:], -SHIFT)

        # PE pstate warm-up while the weights are in flight; result never read.
        warm_ps = ps.tile([128, 512], F32, tag="sb", bufs=2)
        for i in range(16):
            nc.tensor.matmul(
                warm_ps[:, 0:128], ones16[:], ones16[:],
                start=(i == 0), stop=(i == 15),
            )

        # Weights gate all compute: k=0 chunks on the scalar ring, k=1 on sync.
        hwT, wts = [], []
        for k, ring in ((0, nc.scalar), (1, nc.sync)):
            t = const.tile([128, HD], F16, tag=f"hwT{k}")
            ring.dma_start(t[:], hwTd[k * 128:(k + 1) * 128, :])
            hwT.append(t)
            t2 = wpool.tile([128, BPC, M], F16, tag=f"wts{k}")
            ring.dma_start(
                t2[:], Wd[:, k * 128:(k + 1) * 128, :].rearrange("b p m -> p b m")
            )
            wts.append(t2)

        # wp[j][:, b, :] = (head_w @ W[b]) chunk j   (fp16 lhsT for scores)
        wp_flat = []
        for j in range(HC):
            acc = ps.tile([128, BPC * M], F32, tag="cc", bufs=2)
            for k in range(2):
                nc.tensor.matmul(
                    acc[:],
                    hwT[k][:, j * 128:(j + 1) * 128],
                    wts[k][:].rearrange("p b m -> p (b m)"),
                    start=(k == 0),
                    stop=(k == 1),
                )
            t = wpool.tile([128, BPC, M], F16, tag=f"wp{j}")
            nc.vector.tensor_copy(t[:].rearrange("p b m -> p (b m)"), acc[:])
            wp_flat.append(t)

        # wpT[b] = Wp[b].T as [128m, 512h] fp16 (lhsT for all C matmuls).
        wpT16_all = [None] * BPC

        def make_wpT(b):
            wpT_ps = ps.tile([128, HD], F32, tag="cc", bufs=2)
            for k in range(2):
                nc.tensor.matmul(
                    wpT_ps[:, 0:HD], wts[k][:, b, :], hwT[k][:],
                    start=(k == 0), stop=(k == 1),
                )
            wpT16 = wpool.tile([128, HD], BF16, tag=f"wpT16_{b}")
            nc.scalar.copy(wpT16[:], wpT_ps[:, 0:HD])
            wpT16_all[b] = wpT16

        make_wpT(0)

        # --- steady state: software-pipelined subtile stream ---
        mtiles = [(b, mt) for b in range(BPC) for mt in range(NMT)]
        subtiles = [(k, s) for k in range(len(mtiles)) for s in range(NSUB)]
        LAST = len(mtiles) - 1
        h_tiles = [None] * len(mtiles)

        # Ring discipline: the scalar ring must NEVER have a load queued
        # behind a store (a store waits on compute; head-of-line blocking
        # would starve the load stream and the whole pipeline oscillates).
        # - mtiles 0-2 load split across both rings (sync is store-free
        #   that early), mtile 0 in per-subtile slices for a fast start.
        # - mtiles 3-7 load as whole 2MB pieces on the scalar ring only.
        # - stores ride the sync ring; once the last load has been EMITTED
        #   (iteration (3,0) emits load_mtile(7)), stores of mtiles >= 5
        #   alternate across both rings so the final drain runs at the
        #   two-queue aggregate rate instead of one queue's ~250GB/s.
        def load_mtile(k, mode):
            b, mt = mtiles[k]
            n0 = mt * NTL
            h = hpool.tile([128, HC, NTL], F16, tag="h")
            if mode == "slices":
                for q in range(NSUB):
                    q0 = q * NT
                    nc.scalar.dma_start(
                        h[:, 0:2, q0:q0 + NT],
                        Hd[b, 0:256, n0 + q0:n0 + q0 + NT].rearrange(
                            "(c p) n -> p c n", p=128),
                    )
                    nc.sync.dma_start(
                        h[:, 2:4, q0:q0 + NT],
                        Hd[b, 256:512, n0 + q0:n0 + q0 + NT].rearrange(
                            "(c p) n -> p c n", p=128),
                    )
            elif mode == "split":
                nc.scalar.dma_start(
                    h[:, 0:2, :],
                    Hd[b, 0:256, n0:n0 + NTL].rearrange("(c p) n -> p c n", p=128),
                )
                nc.sync.dma_start(
                    h[:, 2:4, :],
                    Hd[b, 256:512, n0:n0 + NTL].rearrange("(c p) n -> p c n", p=128),
                )
            else:
                nc.scalar.dma_start(
                    h[:], Hd[b, :, n0:n0 + NTL].rearrange("(c p) n -> p c n", p=128)
                )
            h_tiles[k] = h

        load_mtile(0, "slices")
        load_mtile(1, "split")
        load_mtile(2, "split")
        load_mtile(3, "whole")

        # Two-deep software pipeline.  Iteration t issues, in PE order:
        #   scores(t) | C-chunks 2-3 of (t-2) from normalized A |
        #   sum(t-1), C-chunks 0-1 of (t-1) from raw E.
        pend1 = None  # (k, s, e)    awaiting sum/recip/a/C01
        pend2 = None  # (k, s, a)    awaiting C23 + evac
        c_tiles = [None] * len(mtiles)
        for t in range(len(subtiles) + 2):
            if t < len(subtiles):
                k, s = subtiles[t]
                if s == 0 and k + 4 < len(mtiles):
                    load_mtile(k + 4, "whole")
                b, mt = mtiles[k]
                # emit batch b+1's projection at a slack slot well before
                # its first subtile
                if s == 2 and mt == NMT - 1 and b + 1 < BPC:
                    make_wpT(b + 1)
                s0 = s * NT
                sc = ps.tile([128, NT], F32, tag="sc", bufs=2)
                for j in range(HC):
                    nc.tensor.matmul(
                        sc[:], wp_flat[j][:, b, :], h_tiles[k][:, j, s0:s0 + NT],
                        start=(j == 0), stop=(j == HC - 1),
                    )
                e = epool.tile([128, NT], BF16, tag="e")
                nc.scalar.activation(
                    e[:], sc[:], mybir.ActivationFunctionType.Exp,
                    bias=neg_shift[:], scale=1.0,
                )
                this1 = (k, s, e)
            else:
                this1 = None

            # C23 matmuls of (t-2) first: their inputs are the oldest in
            # flight, so the PE never waits here
            if pend2 is not None:
                k, s, a = pend2
                b, mt = mtiles[k]
                s0 = s * NT
                c_cur = c_tiles[k]
                wpT16 = wpT16_all[b]
                c_ps = ps.tile([128, 2 * NT], F32, tag="cc", bufs=2)
                for j in range(2, HC):
                    nc.tensor.matmul(
                        c_ps[:, (j - 2) * NT:(j - 1) * NT],
                        wpT16[:, j * 128:(j + 1) * 128], a[:],
                    )
                nc.scalar.copy(
                    c_cur[:, 2:4, s0:s0 + NT],
                    c_ps[:].rearrange("p (c n) -> p c n", c=2),
                )
                n0 = mt * NTL
                if k < LAST and s == NSUB - 1:
                    # chunks 2-3 of the whole mtile: 1MB, 4KB HBM runs.
                    # 2KB-run stores lose the per-packet SDMA round-robin
                    # 2:1 against the 4KB-run loads and the store stream
                    # falls behind compute -> c-tile WAR stalls -> PE cold.
                    nc.sync.dma_start(
                        Od[b, 256:512, n0:n0 + NTL].rearrange(
                            "(c p) n -> p c n", p=128),
                        c_cur[:, 2:4, :],
                    )
                elif k == LAST and s % 2 == 1:
                    nh0 = n0 + (s - 1) * NT
                    ch0 = (s - 1) * NT
                    nc.sync.dma_start(
                        Od[b, 256:512, nh0:nh0 + 2 * NT].rearrange(
                            "(c p) n -> p c n", p=128),
                        c_cur[:, 2:4, ch0:ch0 + 2 * NT],
                    )

            if pend1 is not None:
                k, s, e = pend1
                b, mt = mtiles[k]
                s0 = s * NT
                if s == 0:
                    c_new = cpool.tile([128, HC, NTL], F16, tag="c_full")
                    c_tiles[k] = c_new
                c_cur = c_tiles[k]
                # S broadcast to every partition in one matmul
                sb = ps.tile([128, NT], F32, tag="sb", bufs=2)
                nc.tensor.matmul(sb[:], ones16[:], e[:])
                r = rpool.tile([128, NT], F32, tag="r")
                nc.vector.reciprocal_approx_fast(r[:], sb[:])
                a = apool.tile([128, NT], BF16, tag="a")
                nc.gpsimd.tensor_mul(a[:], e[:], r[:])
                wpT16 = wpT16_all[b]
                c_ps = ps.tile([128, 2 * NT], F32, tag="cc", bufs=2)
                for j in range(2):
                    nc.tensor.matmul(
                        c_ps[:, j * NT:(j + 1) * NT],
                        wpT16[:, j * 128:(j + 1) * 128], e[:],
                    )
                nc.vector.tensor_mul(
                    c_cur[:, 0:2, s0:s0 + NT],
                    c_ps[:].rearrange("p (c n) -> p c n", c=2),
                    r[:].rearrange("p (o n) -> p o n", o=1).broadcast_to(
                        [128, 2, NT]),
                )
                n0 = mt * NTL
                if k < LAST and s == NSUB - 1:
                    # chunks 0-1 of the whole mtile: 1MB, 4KB HBM runs
                    nc.sync.dma_start(
                        Od[b, 0:256, n0:n0 + NTL].rearrange(
                            "(c p) n -> p c n", p=128),
                        c_cur[:, 0:2, :],
                    )
                elif k == LAST and s % 2 == 1:
                    nh0 = n0 + (s - 1) * NT
                    ch0 = (s - 1) * NT
                    nc.sync.dma_start(
                        Od[b, 0:256, nh0:nh0 + 2 * NT].rearrange(
                            "(c p) n -> p c n", p=128),
                        c_cur[:, 0:2, ch0:ch0 + 2 * NT],
                    )
                this2 = (k, s, a)
            else:
                this2 = None

            pend2 = this2
            pend1 = this1

    nc.compile()
    return nc


_NC = None


def _get_nc():
    global _NC
    if _NC is None:
        _NC = build_nc()
    return _NC


def kernel(H, W, head_w, _run_kwargs=None):
    assert H.shape == (B, HD, HH, WW) and W.shape == (B, TD, M)
    assert head_w.shape == (HD, TD)
    nc = _get_nc()

    Hf = np.ascontiguousarray(H, dtype=np.float32).reshape(B, HD, N).astype(np.float16)
    Wc = np.ascontiguousarray(W, dtype=np.float32).astype(np.float16)
    hwT = np.ascontiguousarray(head_w.T, dtype=np.float32).astype(np.float16)

    in_maps = [
        {
            "H": Hf[i * BPC:(i + 1) * BPC],
            "W": Wc[i * BPC:(i + 1) * BPC],
            "head_wT": hwT,
        }
        for i in range(NCORES)
    ]
    res = run_bass_kernel_spmd(
        nc, in_maps, core_ids=list(range(NCORES)), **(_run_kwargs or {})
    )
    out = np.concatenate([res.results[i]["out"] for i in range(NCORES)], axis=0)
    if _run_kwargs:
        kernel.last_results = res
    return out.reshape(B, HD, HH, WW).astype(np.float32)


# revision 30
# speedup vs baseline: 1.1988x; 1.1988x over previous
"""Trainium2 Bass kernel: batched 1x1-conv projection + attention-style softmax mixing.

Reference computation (per batch b):
    Wp     = head_w @ W[b]                  # [512, 128]
    scores = Hf[b].T @ Wp                   # [4096, 128]   (Hf = H reshaped [512, 4096])
    A      = softmax(scores, axis=1)        # over M=128
    C      = A @ Wp.T                       # [4096, 512]
    out[b] = C.T                            # [512, 4096] -> [512, 64, 64]

Sharding: data-parallel over batch B=32 across 8 NeuronCores (4 batches/core).

Per core 16.8MB of H in + 16.8MB of C out; the 16 SDMA engines cap at
~26GB/s each with 4KB HBM runs (~416GB/s aggregate), so the DMA stream is
the end-to-end floor: ~34MB / 416GB/s ~ 82us + ~7us fixed engine preamble.
The schedule keeps both HWDGE rings (scalar + sync) loaded symmetrically
the whole run so neither stream straggles:

  - Every H macro-tile (2MB) is split h-chunkwise: chunks 0-1 load on the
    scalar ring, chunks 2-3 on the sync ring (1MB each, 4KB runs).
  - Every C macro-tile stores as two 1MB pieces at mtile end: chunks 0-1
    on the sync ring, chunks 2-3 on the scalar ring -- full 2048-element
    rows, i.e. 4KB HBM runs (1024-wide pieces would halve the per-engine
    packet rate).  Each ring therefore carries 1MB load + 1MB store per
    mtile period; loads and stores never compete within a ring FIFO
    because a store's data is always ready well before the ring drains
    to it.  The last mtile's stores split into 512KB n-halves issued as
    soon as each half is evacuated, to shrink the final flush.
  - All matmul operands are fp16 (no fp32r anywhere): FWL stays enabled,
    LDWEIGHTS overlaps the matmul stream, and PE issue stays ~2us/subtile.
  - Softmax normalization is deferred: C_raw = E @ WpT for chunks 0-1 with
    the 1/S multiply fused into the DVE PSUM evacuation; chunks 2-3 use
    A = E*r from the gpsimd (which cannot read PSUM) and evacuate as one
    [128,1024] scalar-engine copy spanning both PSUM banks.
  - PSUM: tags sc(2 banks)+sb(2)+cc(4) = 8 banks; projections ride the
    cc rotation at batch boundaries.
"""

import numpy as np

from concourse import bacc, mybir, tile
from concourse.bass_utils import run_bass_kernel_spmd

B, HD, HH, WW = 32, 512, 64, 64
TD, M = 256, 128
N = HH * WW          # 4096
NCORES = 8
BPC = B // NCORES    # 4 batches per core
NT = 512             # n-tile (free dim per matmul, bounded by one PSUM bank)
NTL = 2048           # n-macro-tile per DMA transfer (4KB strips in HBM)
NMT = N // NTL       # 2 macro-tiles per batch
NSUB = NTL // NT     # 4 matmul subtiles per macro-tile
HC = HD // 128       # 4 h-chunks
SHIFT = 64.0         # softmax stabilization shift

F32 = mybir.dt.float32
F16 = mybir.dt.float16
BF16 = mybir.dt.bfloat16


def build_nc():
    from contextlib import ExitStack

    nc = bacc.Bacc("TRN2", target_bir_lowering=False, debug=False, num_devices=NCORES)
    Hd = nc.dram_tensor("H", [BPC, HD, N], F16, kind="ExternalInput").ap()
    # wk{k} = [head_wT chunk k | W chunk k flattened], packed on host
    Wkd = [
        nc.dram_tensor(f"wk{k}", [128, HD + BPC * M], F16, kind="ExternalInput").ap()
        for k in range(2)
    ]
    Od = nc.dram_tensor("out", [BPC, HD, N], F16, kind="ExternalOutput").ap()

    with tile.TileContext(nc) as tc, ExitStack() as ctx:
        const = ctx.enter_context(tc.tile_pool(name="const", bufs=1))
        wpool = ctx.enter_context(tc.tile_pool(name="wp", bufs=1))
        hpool = ctx.enter_context(tc.tile_pool(name="h", bufs=5))
        epool = ctx.enter_context(tc.tile_pool(name="e", bufs=2))
        apool = ctx.enter_context(tc.tile_pool(name="a", bufs=3))
        cpool = ctx.enter_context(tc.tile_pool(name="c", bufs=4))
        rpool = ctx.enter_context(tc.tile_pool(name="r", bufs=2))
        ps = ctx.enter_context(tc.tile_pool(name="ps", bufs=1, space="PSUM"))

        # e/a/wpT ride bf16: exp values span ~e^-44..e^2, far beyond fp16's
        # dynamic range (fp16 e underflows to S=0 -> 1/S=inf -> NaN), while
        # bf16 keeps the fp32 exponent.  The precision-critical scores path
        # (h, wp_flat) stays fp16.
        ones16 = const.tile([128, 128], BF16, tag="ones16")
        nc.vector.memset(ones16[:], 1.0)
        neg_shift = const.tile([128, 1], F32, tag="neg_shift")
        nc.vector.memset(neg_shift[:], -SHIFT)

        # PE pstate warm-up bridging the weight wait (~6.5->10us): keeps the
        # PE HAM-busy so projections run at 2.4GHz; result never read.
        warm_ps = ps.tile([128, 512], F32, tag="sb", bufs=2)
        for i in range(32):
            nc.tensor.matmul(
                warm_ps[:, 0:128], ones16[:], ones16[:],
                start=(i == 0), stop=(i == 31),
            )

        # Weights gate all compute.  head_wT chunk k and W chunk k are packed
        # host-side into one [128, 1024] tensor per k: one DMA per ring, one
        # completion round-trip.
        wk = []
        for k, ring in ((0, nc.scalar), (1, nc.sync)):
            t = const.tile([128, HD + BPC * M], F16, tag=f"wk{k}")
            ring.dma_start(t[:], Wkd[k])
            wk.append(t)
        hwT = [wk[k][:, 0:HD] for k in range(2)]
        wts_flat = [wk[k][:, HD:HD + BPC * M] for k in range(2)]

        # wp[j][:, b, :] = (head_w @ W[b]) chunk j   (fp16 lhsT for scores)
        wp_flat = []
        for j in range(HC):
            acc = ps.tile([128, BPC * M], F32, tag="cc", bufs=2)
            for k in range(2):
                nc.tensor.matmul(
                    acc[:],
                    hwT[k][:, j * 128:(j + 1) * 128],
                    wts_flat[k],
                    start=(k == 0),
                    stop=(k == 1),
                )
            t = wpool.tile([128, BPC, M], F16, tag=f"wp{j}")
            nc.vector.tensor_copy(t[:].rearrange("p b m -> p (b m)"), acc[:])
            wp_flat.append(t)

        # wpT[b] = Wp[b].T as [128m, 512h] bf16 (lhsT for all C matmuls).
        wpT16_all = [None] * BPC

        def make_wpT(b):
            wpT_ps = ps.tile([128, HD], F32, tag="cc", bufs=2)
            for k in range(2):
                nc.tensor.matmul(
                    wpT_ps[:, 0:HD],
                    wts_flat[k][:, b * M:(b + 1) * M], hwT[k],
                    start=(k == 0), stop=(k == 1),
                )
            wpT16 = wpool.tile([128, HD], BF16, tag=f"wpT16_{b}")
            nc.scalar.copy(wpT16[:], wpT_ps[:, 0:HD])
            wpT16_all[b] = wpT16

        make_wpT(0)

        # --- steady state: software-pipelined subtile stream ---
        mtiles = [(b, mt) for b in range(BPC) for mt in range(NMT)]
        subtiles = [(k, s) for k in range(len(mtiles)) for s in range(NSUB)]
        LAST = len(mtiles) - 1
        h_tiles = [None] * len(mtiles)

        # Ring discipline: the scalar ring must NEVER have a load queued
        # behind a store (a store waits on compute; head-of-line blocking
        # would starve the load stream and the whole pipeline oscillates).
        # - mtiles 0-2 load split across both rings (sync is store-free
        #   that early), mtile 0 in per-subtile slices for a fast start.
        # - mtiles 3-7 load as whole 2MB pieces on the scalar ring only.
        # - stores ride the sync ring; once the last load has been EMITTED
        #   (iteration (3,0) emits load_mtile(7)), stores of mtiles >= 5
        #   alternate across both rings so the final drain runs at the
        #   two-queue aggregate rate instead of one queue's ~250GB/s.
        def load_mtile(k, mode):
            b, mt = mtiles[k]
            n0 = mt * NTL
            h = hpool.tile([128, HC, NTL], F16, tag="h")
            if mode == "slices":
                # small first slice so subtile 0 starts early, remainder as
                # one piece per ring (DMA issues cost ~650ns of engine time)
                for q0, q1 in ((0, NT), (NT, NTL)):
                    nc.scalar.dma_start(
                        h[:, 0:2, q0:q1],
                        Hd[b, 0:256, n0 + q0:n0 + q1].rearrange(
                            "(c p) n -> p c n", p=128),
                    )
                    nc.sync.dma_start(
                        h[:, 2:4, q0:q1],
                        Hd[b, 256:512, n0 + q0:n0 + q1].rearrange(
                            "(c p) n -> p c n", p=128),
                    )
            elif mode == "split":
                nc.scalar.dma_start(
                    h[:, 0:2, :],
                    Hd[b, 0:256, n0:n0 + NTL].rearrange("(c p) n -> p c n", p=128),
                )
                nc.sync.dma_start(
                    h[:, 2:4, :],
                    Hd[b, 256:512, n0:n0 + NTL].rearrange("(c p) n -> p c n", p=128),
                )
            else:
                nc.scalar.dma_start(
                    h[:], Hd[b, :, n0:n0 + NTL].rearrange("(c p) n -> p c n", p=128)
                )
            h_tiles[k] = h

        load_mtile(0, "slices")
        load_mtile(1, "split")
        load_mtile(2, "split")
        load_mtile(3, "whole")

        # Two-deep software pipeline.  Iteration t issues, in PE order:
        #   scores(t) | C-chunks 2-3 of (t-2) from normalized A |
        #   sum(t-1), C-chunks 0-1 of (t-1) from raw E.
        pend1 = None  # (k, s, e)    awaiting sum/recip/a/C01
        pend2 = None  # (k, s, a)    awaiting C23 + evac
        c_tiles = [None] * len(mtiles)
        for t in range(len(subtiles) + 2):
            if t < len(subtiles):
                k, s = subtiles[t]
                if s == 0 and k + 4 < len(mtiles):
                    load_mtile(k + 4, "whole")
                b, mt = mtiles[k]
                # emit the remaining batches' projections in early ramp
                # slack (the pipeline is DMA-starved there anyway); at the
                # old batch-boundary slots they cost ~2.5us of pipeline
                # disruption each
                if s == 1 and k < 3:
                    make_wpT(k + 1)
                s0 = s * NT
                sc = ps.tile([128, NT], F32, tag="sc", bufs=2)
                for j in range(HC):
                    nc.tensor.matmul(
                        sc[:], wp_flat[j][:, b, :], h_tiles[k][:, j, s0:s0 + NT],
                        start=(j == 0), stop=(j == HC - 1),
                    )
                e = epool.tile([128, NT], BF16, tag="e")
                nc.scalar.activation(
                    e[:], sc[:], mybir.ActivationFunctionType.Exp,
                    bias=neg_shift[:], scale=1.0,
                )
                this1 = (k, s, e)
            else:
                this1 = None

            # C23 matmuls of (t-2) first: their inputs are the oldest in
            # flight, so the PE never waits here
            if pend2 is not None:
                k, s, a = pend2
                b, mt = mtiles[k]
                s0 = s * NT
                c_cur = c_tiles[k]
                wpT16 = wpT16_all[b]
                c_ps = ps.tile([128, 2 * NT], F32, tag="cc", bufs=2)
                for j in range(2, HC):
                    nc.tensor.matmul(
                        c_ps[:, (j - 2) * NT:(j - 1) * NT],
                        wpT16[:, j * 128:(j + 1) * 128], a[:],
                    )
                nc.scalar.copy(
                    c_cur[:, 2:4, s0:s0 + NT],
                    c_ps[:].rearrange("p (c n) -> p c n", c=2),
                )
                n0 = mt * NTL
                if k < LAST and s == NSUB - 1:
                    # chunks 2-3 of the whole mtile: 1MB, 4KB HBM runs.
                    # 2KB-run stores lose the per-packet SDMA round-robin
                    # 2:1 against the 4KB-run loads and the store stream
                    # falls behind compute -> c-tile WAR stalls -> PE cold.
                    nc.sync.dma_start(
                        Od[b, 256:512, n0:n0 + NTL].rearrange(
                            "(c p) n -> p c n", p=128),
                        c_cur[:, 2:4, :],
                    )
                elif k == LAST and s % 2 == 1:
                    # tail: scalar ring is load-free by now; alternate so the
                    # final flush drains at the two-queue rate
                    nh0 = n0 + (s - 1) * NT
                    ch0 = (s - 1) * NT
                    nc.scalar.dma_start(
                        Od[b, 256:512, nh0:nh0 + 2 * NT].rearrange(
                            "(c p) n -> p c n", p=128),
                        c_cur[:, 2:4, ch0:ch0 + 2 * NT],
                    )

            if pend1 is not None:
                k, s, e = pend1
                b, mt = mtiles[k]
                s0 = s * NT
                if s == 0:
                    c_new = cpool.tile([128, HC, NTL], F16, tag="c_full")
                    c_tiles[k] = c_new
                c_cur = c_tiles[k]
                # S broadcast to every partition in one matmul
                sb = ps.tile([128, NT], F32, tag="sb", bufs=2)
                nc.tensor.matmul(sb[:], ones16[:], e[:])
                r = rpool.tile([128, NT], F32, tag="r")
                nc.vector.reciprocal_approx_fast(r[:], sb[:])
                a = apool.tile([128, NT], BF16, tag="a")
                nc.gpsimd.tensor_mul(a[:], e[:], r[:])
                wpT16 = wpT16_all[b]
                c_ps = ps.tile([128, 2 * NT], F32, tag="cc", bufs=2)
                for j in range(2):
                    nc.tensor.matmul(
                        c_ps[:, j * NT:(j + 1) * NT],
                        wpT16[:, j * 128:(j + 1) * 128], e[:],
                    )
                nc.vector.tensor_mul(
                    c_cur[:, 0:2, s0:s0 + NT],
                    c_ps[:].rearrange("p (c n) -> p c n", c=2),
                    r[:].rearrange("p (o n) -> p o n", o=1).broadcast_to(
                        [128, 2, NT]),
                )
                n0 = mt * NTL
                if k < LAST and s == NSUB - 1:
                    # chunks 0-1 of the whole mtile: 1MB, 4KB HBM runs
                    nc.sync.dma_start(
                        Od[b, 0:256, n0:n0 + NTL].rearrange(
                            "(c p) n -> p c n", p=128),
                        c_cur[:, 0:2, :],
                    )
                elif k == LAST and s % 2 == 1:
                    nh0 = n0 + (s - 1) * NT
                    ch0 = (s - 1) * NT
                    nc.sync.dma_start(
                        Od[b, 0:256, nh0:nh0 + 2 * NT].rearrange(
                            "(c p) n -> p c n", p=128),
                        c_cur[:, 0:2, ch0:ch0 + 2 * NT],
                    )
                this2 = (k, s, a)
            else:
                this2 = None

            pend2 = this2
            pend1 = this1

    nc.compile()
    return nc


_NC = None


def _get_nc():
    global _NC
    if _NC is None:
        _NC = build_nc()
    return _NC


def kernel(H, W, head_w, _run_kwargs=None):
    assert H.shape == (B, HD, HH, WW) and W.shape == (B, TD, M)
    assert head_w.shape == (HD, TD)
    nc = _get_nc()

    Hf = np.ascontiguousarray(H, dtype=np.float32).reshape(B, HD, N).astype(np.float16)
    Wc = np.asarray(W, dtype=np.float32).astype(np.float16)
    hwT = np.asarray(head_w.T, dtype=np.float32).astype(np.float16)

    in_maps = []
    for i in range(NCORES):
        Wcore = Wc[i * BPC:(i + 1) * BPC]  # [BPC, TD, M]
        m = {"H": Hf[i * BPC:(i + 1) * BPC]}
        for k in range(2):
            wts = np.ascontiguousarray(
                Wcore[:, k * 128:(k + 1) * 128, :].transpose(1, 0, 2)
            ).reshape(128, BPC * M)
            m[f"wk{k}"] = np.ascontiguousarray(
                np.concatenate([hwT[k * 128:(k + 1) * 128, :], wts], axis=1)
            )
        in_maps.append(m)
    res = run_bass_kernel_spmd(
        nc, in_maps, core_ids=list(range(NCORES)), **(_run_kwargs or {})
    )
    out = np.concatenate([res.results[i]["out"] for i in range(NCORES)], axis=0)
    if _run_kwargs:
        kernel.last_results = res
    return out.reshape(B, HD, HH, WW).astype(np.float32)


# revision 34
# speedup vs baseline: 1.2356x; 1.0308x over previous
"""Trainium2 Bass kernel: batched 1x1-conv projection + attention-style softmax mixing.

Reference computation (per batch b):
    Wp     = head_w @ W[b]                  # [512, 128]
    scores = Hf[b].T @ Wp                   # [4096, 128]   (Hf = H reshaped [512, 4096])
    A      = softmax(scores, axis=1)        # over M=128
    C      = A @ Wp.T                       # [4096, 512]
    out[b] = C.T                            # [512, 4096] -> [512, 64, 64]

Sharding: data-parallel over batch B=32 across 8 NeuronCores (4 batches/core).

Per core 16.8MB of H in + 16.8MB of C out; the 16 SDMA engines cap at
~26GB/s each with 4KB HBM runs (~416GB/s aggregate), so the DMA stream is
the end-to-end floor: ~34MB / 416GB/s ~ 82us + ~7us fixed engine preamble.
The schedule keeps both HWDGE rings (scalar + sync) loaded symmetrically
the whole run so neither stream straggles:

  - Every H macro-tile (2MB) is split h-chunkwise: chunks 0-1 load on the
    scalar ring, chunks 2-3 on the sync ring (1MB each, 4KB runs).
  - Every C macro-tile stores as two 1MB pieces at mtile end: chunks 0-1
    on the sync ring, chunks 2-3 on the scalar ring -- full 2048-element
    rows, i.e. 4KB HBM runs (1024-wide pieces would halve the per-engine
    packet rate).  Each ring therefore carries 1MB load + 1MB store per
    mtile period; loads and stores never compete within a ring FIFO
    because a store's data is always ready well before the ring drains
    to it.  The last mtile's stores split into 512KB n-halves issued as
    soon as each half is evacuated, to shrink the final flush.
  - All matmul operands are fp16 (no fp32r anywhere): FWL stays enabled,
    LDWEIGHTS overlaps the matmul stream, and PE issue stays ~2us/subtile.
  - Softmax normalization is deferred: C_raw = E @ WpT for chunks 0-1 with
    the 1/S multiply fused into the DVE PSUM evacuation; chunks 2-3 use
    A = E*r from the gpsimd (which cannot read PSUM) and evacuate as one
    [128,1024] scalar-engine copy spanning both PSUM banks.
  - PSUM: tags sc(2 banks)+sb(2)+cc(4) = 8 banks; projections ride the
    cc rotation at batch boundaries.
"""

import numpy as np

from concourse import bacc, mybir, tile
from concourse.bass_utils import run_bass_kernel_spmd

B, HD, HH, WW = 32, 512, 64, 64
TD, M = 256, 128
N = HH * WW          # 4096
NCORES = 8
BPC = B // NCORES    # 4 batches per core
NT = 512             # n-tile (free dim per matmul, bounded by one PSUM bank)
NTL = 2048           # n-macro-tile per DMA transfer (4KB strips in HBM)
NMT = N // NTL       # 2 macro-tiles per batch
NSUB = NTL // NT     # 4 matmul subtiles per macro-tile
HC = HD // 128       # 4 h-chunks
SHIFT = 64.0         # softmax stabilization shift

F32 = mybir.dt.float32
F16 = mybir.dt.float16
BF16 = mybir.dt.bfloat16


def build_nc():
    from contextlib import ExitStack

    nc = bacc.Bacc("TRN2", target_bir_lowering=False, debug=False, num_devices=NCORES)
    Hd = nc.dram_tensor("H", [BPC, HD, N], F16, kind="ExternalInput").ap()
    # wk{k} = [head_wT chunk k | W chunk k flattened], packed on host
    Wkd = [
        nc.dram_tensor(f"wk{k}", [128, HD + BPC * M], F16, kind="ExternalInput").ap()
        for k in range(2)
    ]
    Od = nc.dram_tensor("out", [BPC, HD, N], F16, kind="ExternalOutput").ap()

    with tile.TileContext(nc) as tc, ExitStack() as ctx:
        const = ctx.enter_context(tc.tile_pool(name="const", bufs=1))
        wpool = ctx.enter_context(tc.tile_pool(name="wp", bufs=1))
        hpool = ctx.enter_context(tc.tile_pool(name="h", bufs=5))
        epool = ctx.enter_context(tc.tile_pool(name="e", bufs=2))
        apool = ctx.enter_context(tc.tile_pool(name="a", bufs=3))
        cpool = ctx.enter_context(tc.tile_pool(name="c", bufs=4))
        rpool = ctx.enter_context(tc.tile_pool(name="r", bufs=2))
        ps = ctx.enter_context(tc.tile_pool(name="ps", bufs=1, space="PSUM"))

        # e/a/wpT ride bf16: exp values span ~e^-44..e^2, far beyond fp16's
        # dynamic range (fp16 e underflows to S=0 -> 1/S=inf -> NaN), while
        # bf16 keeps the fp32 exponent.  The precision-critical scores path
        # (h, wp_flat) stays fp16.
        ones16 = const.tile([128, 128], BF16, tag="ones16")
        nc.vector.memset(ones16[:], 1.0)
        neg_shift = const.tile([128, 1], F32, tag="neg_shift")
        nc.vector.memset(neg_shift[:], -SHIFT)

        # PE pstate warm-up bridging the weight wait (~6.5->10us): keeps the
        # PE HAM-busy so projections run at 2.4GHz; result never read.
        warm_ps = ps.tile([128, 512], F32, tag="sb", bufs=2)
        for i in range(16):
            nc.tensor.matmul(
                warm_ps[:, 0:128], ones16[:], ones16[:],
                start=(i == 0), stop=(i == 15),
            )

        # Weights gate all compute.  head_wT chunk k and W chunk k are packed
        # host-side into one [128, 1024] tensor per k: one DMA per ring, one
        # completion round-trip.
        wk = []
        for k, ring in ((0, nc.scalar), (1, nc.sync)):
            t = const.tile([128, HD + BPC * M], F16, tag=f"wk{k}")
            ring.dma_start(t[:], Wkd[k])
            wk.append(t)
        hwT = [wk[k][:, 0:HD] for k in range(2)]
        wts_flat = [wk[k][:, HD:HD + BPC * M] for k in range(2)]

        # wp[j][:, b, :] = (head_w @ W[b]) chunk j   (fp16 lhsT for scores)
        wp_flat = []
        for j in range(HC):
            acc = ps.tile([128, BPC * M], F32, tag="cc", bufs=2)
            for k in range(2):
                nc.tensor.matmul(
                    acc[:],
                    hwT[k][:, j * 128:(j + 1) * 128],
                    wts_flat[k],
                    start=(k == 0),
                    stop=(k == 1),
                )
            t = wpool.tile([128, BPC, M], F16, tag=f"wp{j}")
            nc.vector.tensor_copy(t[:].rearrange("p b m -> p (b m)"), acc[:])
            wp_flat.append(t)

        # wpT[b] = Wp[b].T as [128m, 512h] bf16 (lhsT for all C matmuls).
        wpT16_all = [None] * BPC

        def make_wpT(b):
            wpT_ps = ps.tile([128, HD], F32, tag="cc", bufs=2)
            for k in range(2):
                nc.tensor.matmul(
                    wpT_ps[:, 0:HD],
                    wts_flat[k][:, b * M:(b + 1) * M], hwT[k],
                    start=(k == 0), stop=(k == 1),
                )
            wpT16 = wpool.tile([128, HD], BF16, tag=f"wpT16_{b}")
            nc.scalar.copy(wpT16[:], wpT_ps[:, 0:HD])
            wpT16_all[b] = wpT16

        make_wpT(0)

        # --- steady state: software-pipelined subtile stream ---
        mtiles = [(b, mt) for b in range(BPC) for mt in range(NMT)]
        subtiles = [(k, s) for k in range(len(mtiles)) for s in range(NSUB)]
        LAST = len(mtiles) - 1
        h_tiles = [None] * len(mtiles)

        # Ring discipline: the scalar ring must NEVER have a load queued
        # behind a store (a store waits on compute; head-of-line blocking
        # would starve the load stream and the whole pipeline oscillates).
        # - mtiles 0-2 load split across both rings (sync is store-free
        #   that early), mtile 0 in per-subtile slices for a fast start.
        # - mtiles 3-7 load as whole 2MB pieces on the scalar ring only.
        # - stores ride the sync ring; once the last load has been EMITTED
        #   (iteration (3,0) emits load_mtile(7)), stores of mtiles >= 5
        #   alternate across both rings so the final drain runs at the
        #   two-queue aggregate rate instead of one queue's ~250GB/s.
        def load_mtile(k, mode):
            b, mt = mtiles[k]
            n0 = mt * NTL
            h = hpool.tile([128, HC, NTL], F16, tag="h")
            if mode == "slices":
                # per-subtile slices so subtile s can start after 256KB/ring
                for q0, q1 in ((0, NT), (NT, 2 * NT), (2 * NT, 3 * NT),
                               (3 * NT, NTL)):
                    nc.scalar.dma_start(
                        h[:, 0:2, q0:q1],
                        Hd[b, 0:256, n0 + q0:n0 + q1].rearrange(
                            "(c p) n -> p c n", p=128),
                    )
                    nc.sync.dma_start(
                        h[:, 2:4, q0:q1],
                        Hd[b, 256:512, n0 + q0:n0 + q1].rearrange(
                            "(c p) n -> p c n", p=128),
                    )
            elif mode == "split":
                nc.scalar.dma_start(
                    h[:, 0:2, :],
                    Hd[b, 0:256, n0:n0 + NTL].rearrange("(c p) n -> p c n", p=128),
                )
                nc.sync.dma_start(
                    h[:, 2:4, :],
                    Hd[b, 256:512, n0:n0 + NTL].rearrange("(c p) n -> p c n", p=128),
                )
            else:
                nc.scalar.dma_start(
                    h[:], Hd[b, :, n0:n0 + NTL].rearrange("(c p) n -> p c n", p=128)
                )
            h_tiles[k] = h

        load_mtile(0, "slices")
        load_mtile(1, "split")
        load_mtile(2, "split")
        load_mtile(3, "whole")

        # Two-deep software pipeline.  Iteration t issues, in PE order:
        #   scores(t) | C-chunks 2-3 of (t-2) from normalized A |
        #   sum(t-1), C-chunks 0-1 of (t-1) from raw E.
        pend1 = None  # (k, s, e)    awaiting sum/recip/a/C01
        pend2 = None  # (k, s, a)    awaiting C23 + evac
        c_tiles = [None] * len(mtiles)
        for t in range(len(subtiles) + 2):
            if t < len(subtiles):
                k, s = subtiles[t]
                if s == 0 and k + 4 < len(mtiles):
                    load_mtile(k + 4, "whole")
                b, mt = mtiles[k]
                # emit the remaining batches' projections in early ramp
                # slack (the pipeline is DMA-starved there anyway); at the
                # old batch-boundary slots they cost ~2.5us of pipeline
                # disruption each
                if s == 1 and k < 3:
                    make_wpT(k + 1)
                s0 = s * NT
                sc = ps.tile([128, NT], F32, tag="sc", bufs=2)
                for j in range(HC):
                    nc.tensor.matmul(
                        sc[:], wp_flat[j][:, b, :], h_tiles[k][:, j, s0:s0 + NT],
                        start=(j == 0), stop=(j == HC - 1),
                    )
                e = epool.tile([128, NT], BF16, tag="e")
                nc.scalar.activation(
                    e[:], sc[:], mybir.ActivationFunctionType.Exp,
                    bias=neg_shift[:], scale=1.0,
                )
                this1 = (k, s, e)
            else:
                this1 = None

            # C23 matmuls of (t-2) first: their inputs are the oldest in
            # flight, so the PE never waits here
            if pend2 is not None:
                k, s, a = pend2
                b, mt = mtiles[k]
                s0 = s * NT
                c_cur = c_tiles[k]
                wpT16 = wpT16_all[b]
                c_ps = ps.tile([128, 2 * NT], F32, tag="cc", bufs=2)
                for j in range(2, HC):
                    nc.tensor.matmul(
                        c_ps[:, (j - 2) * NT:(j - 1) * NT],
                        wpT16[:, j * 128:(j + 1) * 128], a[:],
                    )
                nc.scalar.copy(
                    c_cur[:, 2:4, s0:s0 + NT],
                    c_ps[:].rearrange("p (c n) -> p c n", c=2),
                )
                n0 = mt * NTL
                if k < LAST and s == NSUB - 1:
                    # chunks 2-3 of the whole mtile: 1MB, 4KB HBM runs.
                    # 2KB-run stores lose the per-packet SDMA round-robin
                    # 2:1 against the 4KB-run loads and the store stream
                    # falls behind compute -> c-tile WAR stalls -> PE cold.
                    nc.sync.dma_start(
                        Od[b, 256:512, n0:n0 + NTL].rearrange(
                            "(c p) n -> p c n", p=128),
                        c_cur[:, 2:4, :],
                    )
                elif k == LAST:
                    # tail: 256KB per subtile on the (now load-free) scalar
                    # ring so the final flush is a trickle, not a 2MB lump
                    nc.scalar.dma_start(
                        Od[b, 256:512, n0 + s0:n0 + s0 + NT].rearrange(
                            "(c p) n -> p c n", p=128),
                        c_cur[:, 2:4, s0:s0 + NT],
                    )

            if pend1 is not None:
                k, s, e = pend1
                b, mt = mtiles[k]
                s0 = s * NT
                if s == 0:
                    c_new = cpool.tile([128, HC, NTL], F16, tag="c_full")
                    c_tiles[k] = c_new
                c_cur = c_tiles[k]
                # S broadcast to every partition in one matmul
                sb = ps.tile([128, NT], F32, tag="sb", bufs=2)
                nc.tensor.matmul(sb[:], ones16[:], e[:])
                r = rpool.tile([128, NT], F32, tag="r")
                nc.vector.reciprocal_approx_fast(r[:], sb[:])
                a = apool.tile([128, NT], BF16, tag="a")
                nc.gpsimd.tensor_mul(a[:], e[:], r[:])
                wpT16 = wpT16_all[b]
                c_ps = ps.tile([128, 2 * NT], F32, tag="cc", bufs=2)
                for j in range(2):
                    nc.tensor.matmul(
                        c_ps[:, j * NT:(j + 1) * NT],
                        wpT16[:, j * 128:(j + 1) * 128], e[:],
                    )
                nc.vector.tensor_mul(
                    c_cur[:, 0:2, s0:s0 + NT],
                    c_ps[:].rearrange("p (c n) -> p c n", c=2),
                    r[:].rearrange("p (o n) -> p o n", o=1).broadcast_to(
                        [128, 2, NT]),
                )
                n0 = mt * NTL
                if k < LAST and s == NSUB - 1:
                    # chunks 0-1 of the whole mtile: 1MB, 4KB HBM runs
                    nc.sync.dma_start(
                        Od[b, 0:256, n0:n0 + NTL].rearrange(
                            "(c p) n -> p c n", p=128),
                        c_cur[:, 0:2, :],
                    )
                elif k == LAST:
                    nc.sync.dma_start(
                        Od[b, 0:256, n0 + s0:n0 + s0 + NT].rearrange(
                            "(c p) n -> p c n", p=128),
                        c_cur[:, 0:2, s0:s0 + NT],
                    )
                this2 = (k, s, a)
            else:
                this2 = None

            pend2 = this2
            pend1 = this1

    nc.compile()
    return nc


_NC = None


def _get_nc():
    global _NC
    if _NC is None:
        _NC = build_nc()
    return _NC


def kernel(H, W, head_w, _run_kwargs=None):
    assert H.shape == (B, HD, HH, WW) and W.shape == (B, TD, M)
    assert head_w.shape == (HD, TD)
    nc = _get_nc()

    Hf = np.ascontiguousarray(H, dtype=np.float32).reshape(B, HD, N).astype(np.float16)
    Wc = np.asarray(W, dtype=np.float32).astype(np.float16)
    hwT = np.asarray(head_w.T, dtype=np.float32).astype(np.float16)

    in_maps = []
    for i in range(NCORES):
        Wcore = Wc[i * BPC:(i + 1) * BPC]  # [BPC, TD, M]
        m = {"H": Hf[i * BPC:(i + 1) * BPC]}
        for k in range(2):
            wts = np.ascontiguousarray(
                Wcore[:, k * 128:(k + 1) * 128, :].transpose(1, 0, 2)
            ).reshape(128, BPC * M)
            m[f"wk{k}"] = np.ascontiguousarray(
                np.concatenate([hwT[k * 128:(k + 1) * 128, :], wts], axis=1)
            )
        in_maps.append(m)
    res = run_bass_kernel_spmd(
        nc, in_maps, core_ids=list(range(NCORES)), **(_run_kwargs or {})
    )
    out = np.concatenate([res.results[i]["out"] for i in range(NCORES)], axis=0)
    if _run_kwargs:
        kernel.last_results = res
    return out.reshape(B, HD, HH, WW).astype(np.float32)
